# revision 1
# baseline (speedup 1.0000x reference)
"""CenterLoss Trainium2 kernel (8-core SPMD, data-parallel over batch).

loss = mean_i( ||feat_i - centers[label_i]|| / count[label_i] )

Device algorithm (per core, batch shard of 2048 rows):
  - radix-100 class factorization: c = 100*h + l
  - one-hot matrices for the local shard: A[i,h]=1[h_i==h], B[i,l]=1[l_i==l]
    (bf16, generated on DVE via is_equal against an iota constant)
  - dist_i = ||feat_i - centers[label_i]||  (dma_gather of center rows,
    DVE subtract, ACT square+accumulate, ACT sqrt)
  - partial histogram  cnt2d[h,l] = sum_i A[i,h] B[i,l]        (PE matmuls)
  - partial dist sums  S2d[h,l]   = sum_i A[i,h] B[i,l] dist_i (PE matmuls)
  - host (the "all-reduce"): cnt = sum_k cnt_k, S = sum_k S_k,
    loss = sum(S / max(cnt,1)) / B
    (exact: sum_i dist_i/count_{label_i} == sum_{h,l} S2d/cnt2d)
"""

from contextlib import ExitStack

import numpy as np

import concourse.bass as bass
import concourse.tile as tile
from concourse import bacc, mybir
from concourse import bass_utils
from concourse.alu_op_type import AluOpType

B, D, C = 16384, 512, 10000
NCORES = 8
BLOC = B // NCORES  # 2048 rows per core
P = 128
TLOC = BLOC // P    # 16 local batch tiles
R = 100             # radix (c = 100*h + l)
DCHUNK = 4          # local tiles per dist DMA chunk
NDC = TLOC // DCHUNK

F32 = mybir.dt.float32
BF16 = mybir.dt.bfloat16
I16 = mybir.dt.int16

_CACHE: dict = {}


def build_program(reps: int = 1):
    """Build + compile the per-core Bass program (SPMD: same program on
    all 8 cores, different input data).

    reps > 1 repeats the whole body, chained through a scalar so DCE keeps
    every rep (for timing: marginal wall-clock per rep = pure device time).
    """
    nc = bacc.Bacc(
        "TRN2", target_bir_lowering=False, debug=False, enable_asserts=False
    )

    feat_d = nc.dram_tensor("feat", [BLOC, D], F32, kind="ExternalInput").ap()
    cent_d = nc.dram_tensor("centers", [C, D], F32, kind="ExternalInput").ap()
    gidx_d = nc.dram_tensor("gidx", [P, BLOC // 16], I16, kind="ExternalInput").ap()
    hloc_d = nc.dram_tensor("hloc", [P, TLOC], I16, kind="ExternalInput").ap()
    lloc_d = nc.dram_tensor("lloc", [P, TLOC], I16, kind="ExternalInput").ap()
    tok_d = nc.dram_tensor("tok", [1, 1], F32, kind="ExternalInput").ap()
    s_out_d = nc.dram_tensor("s_out", [R, R], F32, kind="ExternalOutput").ap()
    c_out_d = nc.dram_tensor("c_out", [R, R], F32, kind="ExternalOutput").ap()

    feat_r = feat_d.rearrange("(p t) d -> p t d", p=P)

    with tile.TileContext(nc) as tc, ExitStack() as ctx:
        const = ctx.enter_context(tc.tile_pool(name="const", bufs=1))
        big = ctx.enter_context(tc.tile_pool(name="big", bufs=5))
        work = ctx.enter_context(tc.tile_pool(name="work", bufs=6))
        fin = ctx.enter_context(tc.tile_pool(name="fin", bufs=2))
        psum = ctx.enter_context(tc.tile_pool(name="psum", bufs=3, space="PSUM"))

        # one-time constant: iota[p, h, j] = h (int16)
        iota_s = const.tile([P, R, TLOC], I16, tag="iota")
        nc.gpsimd.iota(
            iota_s[:], pattern=[[1, R], [0, TLOC]], base=0, channel_multiplier=0
        )

        chain_prev = None
        for _rep in range(reps):
            # ---- small input loads
            hloc_s = const.tile([P, TLOC], I16, tag="hloc")
            nc.sync.dma_start(hloc_s[:], hloc_d[:])
            lloc_s = const.tile([P, TLOC], I16, tag="lloc")
            nc.sync.dma_start(lloc_s[:], lloc_d[:])
            gidx_s = const.tile([P, BLOC // 16], I16, tag="gidx")
            nc.sync.dma_start(gidx_s[:], gidx_d[:])
            tok_s = const.tile([1, 1], F32, tag="tok")
            nc.sync.dma_start(tok_s[:], tok_d[:])

            # ---- local one-hots (bf16): no dist dependency, start early
            hloc_b = hloc_s[:].unsqueeze(1).broadcast_to([P, R, TLOC])
            lloc_b = lloc_s[:].unsqueeze(1).broadcast_to([P, R, TLOC])
            a_loc = fin.tile([P, R, TLOC], BF16, tag="a_loc")
            nc.vector.tensor_tensor(a_loc[:], hloc_b, iota_s[:], AluOpType.is_equal)
            b_loc = fin.tile([P, R, TLOC], BF16, tag="b_loc")
            nc.vector.tensor_tensor(b_loc[:], lloc_b, iota_s[:], AluOpType.is_equal)

            psum_cnt = psum.tile([R, R], F32, tag="psum_cnt")
            for t in range(TLOC):
                nc.tensor.matmul(
                    psum_cnt[:],
                    a_loc[:, :, t],
                    b_loc[:, :, t],
                    start=(t == 0),
                    stop=(t == TLOC - 1),
                )

            # ---- dist path fully pipelined per chunk: DMAs -> sub (DVE) ->
            # square-acc (ACT) -> sqrt -> bf16 -> dist-scaled one-hots ->
            # S matmuls, all on per-chunk tiles so nothing waits on the
            # whole dist vector
            psum_s = psum.tile([R, R], F32, tag="psum_s")
            gcols = (BLOC // 16) // NDC  # gidx columns per chunk
            for q in range(NDC):
                feat_c = big.tile([P, DCHUNK, D], F32, tag="feat")
                nc.sync.dma_start(
                    feat_c[:], feat_r[:, q * DCHUNK : (q + 1) * DCHUNK]
                )
                gath_c = big.tile([P, DCHUNK, D], F32, tag="gath")
                nc.gpsimd.dma_gather(
                    out_ap=gath_c[:],
                    in_ap=cent_d[:],
                    idxs_ap=gidx_s[:, q * gcols : (q + 1) * gcols],
                    num_idxs=BLOC // NDC,
                    num_idxs_reg=BLOC // NDC,
                    elem_size=D,
                    single_packet=False,
                )
                dist2_c = work.tile([P, DCHUNK], F32, tag="d2c")
                for t in range(DCHUNK):
                    diff = work.tile([P, D], F32, tag="diff")
                    nc.vector.tensor_sub(diff[:], feat_c[:, t], gath_c[:, t])
                    sq = work.tile([P, D], F32, tag="sq")
                    nc.scalar.activation(
                        sq[:],
                        diff[:],
                        mybir.ActivationFunctionType.Square,
                        accum_out=dist2_c[:, t : t + 1],
                    )
                dist_bfc = work.tile([P, DCHUNK], BF16, tag="dbfc")
                dist_fc = work.tile([P, DCHUNK], F32, tag="dfc")
                nc.scalar.activation(
                    dist_fc[:], dist2_c[:], mybir.ActivationFunctionType.Sqrt
                )
                nc.vector.tensor_copy(dist_bfc[:], dist_fc[:])
                bp_c = work.tile([P, R, DCHUNK], BF16, tag="bpc")
                nc.vector.tensor_tensor(
                    bp_c[:],
                    b_loc[:, :, q * DCHUNK : (q + 1) * DCHUNK],
                    dist_bfc[:].unsqueeze(1).broadcast_to([P, R, DCHUNK]),
                    AluOpType.mult,
                )
                for t in range(DCHUNK):
                    nc.tensor.matmul(
                        psum_s[:],
                        a_loc[:, :, q * DCHUNK + t],
                        bp_c[:, :, t],
                        start=(q == 0 and t == 0),
                        stop=(q == NDC - 1 and t == DCHUNK - 1),
                    )
            cnt_sb = fin.tile([R, R], F32, tag="cnt_sb")
            nc.vector.tensor_copy(cnt_sb[:], psum_cnt[:])
            s_sb = fin.tile([R, R], F32, tag="s_sb")
            nc.vector.tensor_copy(s_sb[:], psum_s[:])
            # tok/prev chain keeps every rep live under DCE when reps > 1
            # (depends on both result matrices); per-rep work still pipelines
            prev = tok_s if _rep == 0 else chain_prev
            ch1 = fin.tile([1, 1], F32, tag=f"ch1_{_rep}")
            nc.vector.scalar_tensor_tensor(
                out=ch1[:],
                in0=prev[:],
                scalar=0.0,
                in1=s_sb[0:1, 0:1],
                op0=AluOpType.mult,
                op1=AluOpType.add,
            )
            ch2 = fin.tile([1, 1], F32, tag=f"ch2_{_rep}")
            nc.vector.scalar_tensor_tensor(
                out=ch2[:],
                in0=ch1[:],
                scalar=0.0,
                in1=cnt_sb[0:1, 0:1],
                op0=AluOpType.mult,
                op1=AluOpType.add,
            )
            chain_prev = ch2
        # write outputs once (last rep's values + chain dependency)
        nc.sync.dma_start(s_out_d[:], s_sb[:])
        nc.sync.dma_start(c_out_d[:], cnt_sb[:])
        # fold the chain into c_out so every rep stays live
        extra = fin.tile([1, 1], F32, tag="extra")
        nc.vector.scalar_tensor_tensor(
            out=extra[:],
            in0=chain_prev[:],
            scalar=0.0,
            in1=cnt_sb[0:1, 0:1],
            op0=AluOpType.mult,
            op1=AluOpType.add,
        )
        nc.sync.dma_start(c_out_d[0:1, 0:1], extra[:])

    nc.compile()
    return nc


def make_in_maps(feat, label, centers, tok=0.0):
    """Shard + lay out full inputs into the 8 per-core input maps."""
    feat = np.ascontiguousarray(np.asarray(feat, dtype=np.float32))
    label = np.asarray(label, dtype=np.int32)
    centers = np.ascontiguousarray(np.asarray(centers, dtype=np.float32))

    g = np.arange(BLOC)
    perm = (g % P) * TLOC + (g // P)  # gather order -> local row index
    tok_arr = np.full((1, 1), tok, dtype=np.float32)

    in_maps = []
    for k in range(NCORES):
        lab_k = label[k * BLOC : (k + 1) * BLOC]
        gvals = lab_k[perm].astype(np.int16)  # idx list in gather order
        gidx16 = np.ascontiguousarray(gvals.reshape(BLOC // 16, 16).T)  # [16, 128]
        gidx = np.ascontiguousarray(np.tile(gidx16, (P // 16, 1)))
        in_maps.append(
            {
                "feat": feat[k * BLOC : (k + 1) * BLOC],
                "centers": centers,
                "gidx": gidx,
                "hloc": np.ascontiguousarray(
                    (lab_k // R).astype(np.int16).reshape(P, TLOC)
                ),
                "lloc": np.ascontiguousarray(
                    (lab_k % R).astype(np.int16).reshape(P, TLOC)
                ),
                "tok": tok_arr,
            }
        )
    return in_maps


def get_program():
    if "nc" not in _CACHE:
        _CACHE["nc"] = build_program()
    return _CACHE["nc"]


def kernel(feat, label, centers):
    nc = get_program()
    in_maps = make_in_maps(feat, label, centers)
    res = bass_utils.run_bass_kernel_spmd(nc, in_maps, core_ids=list(range(NCORES)))
    s_tot = np.zeros((R, R), dtype=np.float64)
    c_tot = np.zeros((R, R), dtype=np.float64)
    for k in range(NCORES):
        s_tot += res.results[k]["s_out"].astype(np.float64)
        c_tot += res.results[k]["c_out"].astype(np.float64)
    loss = (s_tot / np.maximum(c_tot, 1.0)).sum() / B
    return np.asarray(loss, dtype=np.float32)

